# revision 49
# baseline (speedup 1.0000x reference)
"""GCN2 (8-layer, N=100K, E=1.6M, G=128) Trainium2 Bass kernel, 8-core SPMD.

Strategy (data-parallel over graphs, per sharding hint):
- batch is sorted => graphs are block-contiguous in node ids. 16 graphs/core.
- Per layer: each core computes h for its node shard; shards are AllGathered
  into a full fp16 table [8*NP, 256] in Shared DRAM; the edge aggregation
  A_hat @ h is done per-core over edges grouped by destination:
    * edges sorted into (dst-block of 128 nodes) x (src-range group of <=2*NP
      rows, so gather indices fit int16), padded to chunks of 128 edges
    * dma_gather pulls 128 rows (512B fp16 each) per chunk from the table
    * a one-hot matmul (lhsT = onehot[edge_slot, dst_local], rhs = gathered
      rows) segment-sums each chunk into the block's PSUM accumulator
  The symmetric gcn_norm is folded in: table rows are pre-scaled by
  dinv[src]; dinv[dst]*(1-alpha) is applied per-partition when copying the
  PSUM accumulator out.
- GCN2Conv update: out = a @ W1' + h0 @ W2'' with W1' = b*W1 + (1-b)*I,
  W2'' = ALPHA*(b*W2 + (1-b)*I) folded on host. h0^T is kept resident in
  SBUF (fp16) as the stationary operand; a is transposed on the PE.
- LayerNorm via bn_stats/bn_aggr (fp32), then gamma/beta + ReLU.
- Mean-pool folded into a per-block one-hot matmul on the last layer
  (weights 1/cnt), then a tiny per-core MLP head -> y[16] per core.

The instruction stream must be identical on all 8 cores (SPMD): per-(block,
group) chunk counts are the elementwise MAX across cores; shorter cores pad
with duplicate gather indices and dst_local=-1 (one-hot row of zeros).
"""

import os
import sys
import types
import numpy as np
from contextlib import ExitStack

sys.path.insert(0, "/opt/trn_rl_repo")
if os.path.isdir("/root/.axon_site"):
    sys.path.insert(0, "/root/.axon_site")

import concourse.bass as bass
import concourse.bacc as bacc
import concourse.tile as tile
from concourse import mybir
from concourse.bass_utils import run_bass_kernel_spmd
from concourse.masks import make_identity

f16 = mybir.dt.float16
f32 = mybir.dt.float32
i16 = mybir.dt.int16

# problem constants
N, E, G = 100000, 1600000, 128
D_IN, D_H, L = 771, 256, 8
ALPHA, THETA = 0.1, 0.5
LN_EPS = 1e-5
NCORES = 8
GPC = G // NCORES          # graphs per core
NGROUP = 4                 # src-range groups (int16 gather index limit)
NQUEUES = int(os.environ.get("KQ", "4"))   # SWDGE queues for gathers

LAST_EXEC_NS = None


def _maybe_register_ntff_hook():
    """Register the axon NTFF profiling hook if the image's antenv lacks it."""
    try:
        from antenv import axon_hooks  # noqa: F401
        return
    except ImportError:
        pass
    try:
        from trn_agent_boot.trn_boot import _ntff_profile_via_ctypes
        import antenv
        mod = types.ModuleType("antenv.axon_hooks")
        hook = _ntff_profile_via_ctypes("/opt/axon/libaxon_pjrt.so")
        if hook is None:
            return
        mod.get_axon_ntff_profile_hook = lambda: hook
        sys.modules["antenv.axon_hooks"] = mod
        antenv.axon_hooks = mod
    except Exception:
        pass


# ---------------------------------------------------------------------------
# Host preprocessing
# ---------------------------------------------------------------------------

def preprocess(x, edge_index, batch, lin_in_w, lin_in_b, w1, w2, ln_g, ln_b,
               c1_w, c1_b, c2_w, c2_b, c3_w, c3_b, L_layers=L):
    x = np.asarray(x, np.float32)
    edge_index = np.asarray(edge_index)
    batch = np.asarray(batch)
    n = x.shape[0]
    d_in = x.shape[1]

    # node -> core (graphs are block-contiguous; graph g -> core g // GPC)
    core_of = (batch // GPC).astype(np.int64)
    node_lo = np.searchsorted(batch, np.arange(NCORES) * GPC, side="left")
    node_hi = np.searchsorted(batch, (np.arange(NCORES) + 1) * GPC, side="left")
    shard_sz = node_hi - node_lo
    NP = int(np.ceil(shard_sz.max() / 512.0) * 512)
    NB = NP // 128
    assert NB % 4 == 0
    NR = NB // 4
    # src groups = block ranges; AllGather is split per group so the table
    # distribution for layer l+1 overlaps layer l's block loop.
    GB = 28                               # blocks per group (last group rest)
    GRP_LO = [min(g * GB, NB) for g in range(NGROUP + 1)]
    GRP_NROW = [(GRP_LO[g + 1] - GRP_LO[g]) * 128 for g in range(NGROUP)]
    GROUP_ROWS_L = [NCORES * n for n in GRP_NROW]
    assert max(GROUP_ROWS_L) <= 32767
    AG_BLOCKS = {GRP_LO[g + 1] - 1: g for g in range(NGROUP)}

    # degree includes the self-loop; the loop itself is applied as a
    # per-block rank-1 add in the kernel, not as gathered edges.
    dst_full = np.concatenate([edge_index[1], np.arange(n)]).astype(np.int64)
    deg = np.bincount(dst_full, minlength=n).astype(np.float64)
    dinv = (1.0 / np.sqrt(deg)).astype(np.float32)  # deg >= 1 via self-loop
    src = np.asarray(edge_index[0], np.int64)
    dst = np.asarray(edge_index[1], np.int64)

    ecore = core_of[dst]
    eldst = dst - node_lo[ecore]              # local dst within shard
    eblk = eldst // 128
    edlocal = (eldst % 128).astype(np.int32)
    score = core_of[src]
    slocal = src - node_lo[score]             # local src within shard
    # table rows are in round-major order [round, partition, block%4] so the
    # producer can write hs one contiguous 2KB/partition burst per round.
    sblk = slocal // 128
    sp = slocal % 128
    egrp = np.minimum(sblk // GB, NGROUP - 1).astype(np.int64)
    spos = (sblk // 4 - (GB // 4) * egrp) * 512 + sp * 4 + (sblk % 4)
    eidx16 = (score * np.array(GRP_NROW)[egrp] + spos).astype(np.int32)

    # sort edges by (core, block, group)
    key = ((ecore * NB + eblk) * NGROUP + egrp)
    order = np.argsort(key, kind="stable")
    key_s = key[order]
    idx16_s = eidx16[order]
    dlocal_s = edlocal[order]

    # counts per (core, block, group)
    ncell = NCORES * NB * NGROUP
    cnt = np.bincount(key_s, minlength=ncell).reshape(NCORES, NB, NGROUP)
    # uniform chunk structure: elementwise max across cores
    C_bg = np.ceil(cnt.max(axis=0) / 128.0).astype(np.int64)  # [NB, NGROUP]
    C_b = C_bg.sum(axis=1)                                     # [NB]
    if (C_b == 0).any():
        C_bg[C_b == 0, 0] = 1   # all-pad chunk so the psum accumulator is written
        C_b = C_bg.sum(axis=1)
    TOTCH = int(C_b.sum())

    # segment starts in the sorted edge array
    starts = np.zeros(ncell + 1, np.int64)
    np.cumsum(cnt.reshape(-1), out=starts[1:])

    # --- build per-core padded idx / dst streams --------------------------
    # dst stream (block-major): for b: for g: C_bg[b,g] chunks of 128
    # idx stream  (round/group-major): for r: for g: for b in r: chunks
    dst_cols_off = np.zeros((NB, NGROUP), np.int64)   # col offset of (b,g) in dst stream
    col = 0
    for b in range(NB):
        for g in range(NGROUP):
            dst_cols_off[b, g] = col
            col += C_bg[b, g]
    assert col == TOTCH

    # idx stream offsets per (r, g): columns of 128-idx chunks
    idx_seg_off = np.zeros((NR, NGROUP), np.int64)    # chunk offset of (r,g) seg
    idx_seg_len = np.zeros((NR, NGROUP), np.int64)
    vslot_off = np.zeros((NB, NGROUP), np.int64)      # slot of (b,g) within its (r,g) seg
    ch = 0
    for r in range(NR):
        for g in range(NGROUP):
            idx_seg_off[r, g] = ch
            s = 0
            for b in range(4 * r, 4 * r + 4):
                vslot_off[b, g] = s
                s += C_bg[b, g]
            idx_seg_len[r, g] = s
            ch += s
    assert ch == TOTCH
    CSEG_MAX = int(idx_seg_len.max())
    CSEG_G = [int(idx_seg_len[:, g].max()) for g in range(NGROUP)]

    per_core = []
    for c in range(NCORES):
        idx_stream = np.zeros((TOTCH, 128), np.int16)
        dst_stream = np.full((TOTCH, 128), -1.0, np.float32)
        for b in range(NB):
            for g in range(NGROUP):
                cell = (c * NB + b) * NGROUP + g
                e0, e1 = starts[cell], starts[cell + 1]
                nreal = e1 - e0
                npad = C_bg[b, g] * 128
                vals = np.zeros(npad, np.int16)
                dl = np.full(npad, -1.0, np.float32)
                if nreal > 0:
                    vals[:nreal] = idx16_s[e0:e1].astype(np.int16)
                    vals[nreal:] = vals[nreal - 1]   # duplicate last (row-hit)
                    dl[:nreal] = dlocal_s[e0:e1]
                # chunk layout: chunk j, edge slot p -> stream[row, p]
                vals = vals.reshape(-1, 128)
                dl = dl.reshape(-1, 128)
                # dst stream at block-major cols
                d0 = dst_cols_off[b, g]
                dst_stream[d0:d0 + C_bg[b, g]] = dl
                # idx stream at (r,g)-major cols
                r = b // 4
                i0 = idx_seg_off[r, g] + vslot_off[b, g]
                idx_stream[i0:i0 + C_bg[b, g]] = vals
        # idx DRAM layout for dma_gather: index i of a call at [i%16, i//16].
        # call = contiguous chunk range; within chunk j, slot p: i = j*128+p
        # -> partition (j*128+p)%16 = p%16, column (j*128+p)//16 = j*8 + p//16
        idx_dram = np.zeros((128, TOTCH * 8), np.int16)
        flat = idx_stream.reshape(-1)             # [TOTCH*128]
        ii = np.arange(TOTCH * 128)
        part = (ii % 16).astype(np.int64)
        colx = (ii // 16).astype(np.int64)
        for rep in range(8):
            idx_dram[rep * 16 + part, colx] = flat
        # dst DRAM layout: [128, TOTCH] fp32, partition = edge slot
        dst_dram = dst_stream.T.copy()

        # per-node tables
        nloc = shard_sz[c]
        dv = np.zeros(NP, np.float32)
        dv[:nloc] = dinv[node_lo[c]:node_hi[c]]
        dinvp = (dv * (1.0 - ALPHA)).reshape(NB, 128).T.copy()   # [128, NB]
        dinvr = dv.reshape(NB, 128).T.copy()                     # [128, NB]

        # pooling weights: [128, NB, GPC] fp16, value 1/cnt_graph
        gl = np.zeros(NP, np.int64)
        gl[:nloc] = batch[node_lo[c]:node_hi[c]] - c * GPC
        gcnt = np.bincount(gl[:nloc], minlength=GPC).astype(np.float32)
        gcnt = np.maximum(gcnt, 1.0)
        wp = np.zeros((NP, GPC), np.float32)
        wp[np.arange(nloc), gl[:nloc]] = 1.0 / gcnt[gl[:nloc]]
        wpool = wp.reshape(NB, 128, GPC).transpose(1, 0, 2).copy().astype(np.float16)

        # x^T per block: [NB, 128(feat%128), KT*128(node-major runs)] fp16
        # layout xT[b, p, k*128+n] = x[block b node n, k*128+p] so each
        # partition's bytes are contiguous per block.
        KT = (d_in + 127) // 128
        xpad = np.zeros((NP, KT * 128), np.float16)
        xpad[:nloc, :d_in] = x[node_lo[c]:node_hi[c]].astype(np.float16)
        # [NB, 128n, KT, 128p] -> [NB, 128p, KT, 128n]
        xT = xpad.reshape(NB, 128, KT, 128).transpose(0, 3, 2, 1).copy()
        xT = xT.reshape(NB, 128, KT * 128)

        per_core.append(dict(idx=idx_dram, dstl=dst_dram, dinvp=dinvp,
                             dinvr=dinvr, wpool=wpool, xT=xT))

    # --- shared weights ---------------------------------------------------
    KT = (d_in + 127) // 128
    linw = np.zeros((KT * 128, D_H), np.float16)
    linw[:d_in] = np.asarray(lin_in_w, np.float32).astype(np.float16)
    linw = linw.reshape(KT, 128, D_H)
    linb = np.asarray(lin_in_b, np.float32)

    betas = np.log(THETA / np.arange(1.0, L_layers + 1.0) + 1.0).astype(np.float32)
    I = np.eye(D_H, dtype=np.float32)
    w1p = np.zeros((L_layers, 2, 128, D_H), np.float16)
    w2p = np.zeros((L_layers, 2, 128, D_H), np.float16)
    for l in range(L_layers):
        b_ = betas[l]
        m1 = b_ * np.asarray(w1[l], np.float32) + (1 - b_) * I
        m2 = ALPHA * (b_ * np.asarray(w2[l], np.float32) + (1 - b_) * I)
        w1p[l] = m1.reshape(2, 128, D_H).astype(np.float16)
        w2p[l] = m2.reshape(2, 128, D_H).astype(np.float16)

    shared = dict(
        linw=linw, linb=linb.reshape(1, D_H),
        w1p=w1p, w2p=w2p,
        lng=np.asarray(ln_g, np.float32)[:L_layers],
        lnb=np.asarray(ln_b, np.float32)[:L_layers],
        c1w=np.asarray(c1_w, np.float32).reshape(2, 128, D_H),
        c1b=np.asarray(c1_b, np.float32).reshape(1, D_H),
        c2w=np.asarray(c2_w, np.float32).reshape(2, 128, D_H // 2),
        c2b=np.asarray(c2_b, np.float32).reshape(1, D_H // 2),
        c3w=np.asarray(c3_w, np.float32).reshape(128, 1),
        c3b=np.asarray(c3_b, np.float32).reshape(1, 1),
    )

    meta = dict(NP=NP, NB=NB, NR=NR, KT=KT, L=L_layers, TOTCH=TOTCH,
                GROUP_ROWS_L=GROUP_ROWS_L, AG_BLOCKS=AG_BLOCKS,
                GRP_LO=GRP_LO, CSEG_MAX=CSEG_MAX, CSEG_G=CSEG_G,
                C_bg=C_bg, C_b=C_b, dst_cols_off=dst_cols_off,
                idx_seg_off=idx_seg_off, idx_seg_len=idx_seg_len,
                vslot_off=vslot_off)

    in_maps = []
    for c in range(NCORES):
        m = dict(per_core[c])
        m.update(shared)
        in_maps.append(m)
    return in_maps, meta


# ---------------------------------------------------------------------------
# Bass kernel builder
# ---------------------------------------------------------------------------

def build_kernel(meta):
    dbg = set(os.environ.get("KDBG", "").split(",")) - {""}
    NP, NB, NR, KT = meta["NP"], meta["NB"], meta["NR"], meta["KT"]
    Ll, TOTCH = meta["L"], meta["TOTCH"]
    GROUP_ROWS_L, CSEG_MAX = meta["GROUP_ROWS_L"], meta["CSEG_MAX"]
    CSEG_G = meta["CSEG_G"]
    AG_BLOCKS, GRP_LO = meta["AG_BLOCKS"], meta["GRP_LO"]
    C_bg, C_b = meta["C_bg"], meta["C_b"]
    dst_cols_off = meta["dst_cols_off"]
    idx_seg_off, idx_seg_len = meta["idx_seg_off"], meta["idx_seg_len"]
    vslot_off = meta["vslot_off"]
    CB_MAX = int(C_b.max())
    SRCOLS_MAX = 0
    for r0 in range(0, NR, 4):
        re_ = min(r0 + 4, NR)
        c0 = int(idx_seg_off[r0, 0]) * 8
        c1 = int(idx_seg_off[re_, 0]) * 8 if re_ < NR else TOTCH * 8
        SRCOLS_MAX = max(SRCOLS_MAX, c1 - c0)

    nc = bacc.Bacc("TRN2", target_bir_lowering=False, debug=False,
                   num_devices=NCORES, num_swdge_queues=NQUEUES)

    # external inputs
    t_idx = nc.dram_tensor("idx", [128, TOTCH * 8], i16, kind="ExternalInput")
    t_dst = nc.dram_tensor("dstl", [128, TOTCH], f32, kind="ExternalInput")
    t_dinvp = nc.dram_tensor("dinvp", [128, NB], f32, kind="ExternalInput")
    t_dinvr = nc.dram_tensor("dinvr", [128, NB], f32, kind="ExternalInput")
    t_wpool = nc.dram_tensor("wpool", [128, NB, GPC], f16, kind="ExternalInput")
    t_xT = nc.dram_tensor("xT", [NB, 128, KT * 128], f16, kind="ExternalInput")
    t_linw = nc.dram_tensor("linw", [KT, 128, D_H], f16, kind="ExternalInput")
    t_linb = nc.dram_tensor("linb", [1, D_H], f32, kind="ExternalInput")
    t_w1p = nc.dram_tensor("w1p", [Ll, 2, 128, D_H], f16, kind="ExternalInput")
    t_w2p = nc.dram_tensor("w2p", [Ll, 2, 128, D_H], f16, kind="ExternalInput")
    t_lng = nc.dram_tensor("lng", [Ll, D_H], f32, kind="ExternalInput")
    t_lnb = nc.dram_tensor("lnb", [Ll, D_H], f32, kind="ExternalInput")
    t_c1w = nc.dram_tensor("c1w", [2, 128, D_H], f32, kind="ExternalInput")
    t_c1b = nc.dram_tensor("c1b", [1, D_H], f32, kind="ExternalInput")
    t_c2w = nc.dram_tensor("c2w", [2, 128, D_H // 2], f32, kind="ExternalInput")
    t_c2b = nc.dram_tensor("c2b", [1, D_H // 2], f32, kind="ExternalInput")
    t_c3w = nc.dram_tensor("c3w", [128, 1], f32, kind="ExternalInput")
    t_c3b = nc.dram_tensor("c3b", [1, 1], f32, kind="ExternalInput")
    t_y = nc.dram_tensor("y", [GPC, 1], f32, kind="ExternalOutput")

    def bcast_row(ap_2d, parts=128):
        # [1, D] dram AP -> [parts, D] with 0 partition stride
        return bass.AP(tensor=ap_2d.tensor, offset=ap_2d.offset,
                       ap=[[0, parts]] + list(ap_2d.ap[1:]))

    with tile.TileContext(nc) as tc, ExitStack() as ctx:
        const = ctx.enter_context(tc.tile_pool(name="const", bufs=1))
        resident = ctx.enter_context(tc.tile_pool(name="res", bufs=1))
        work = ctx.enter_context(tc.tile_pool(name="work", bufs=3))
        vpool = ctx.enter_context(tc.tile_pool(name="vpool", bufs=2))
        ohpool = ctx.enter_context(tc.tile_pool(name="ohpool", bufs=2))
        ipool = ctx.enter_context(tc.tile_pool(name="ipool", bufs=2))
        psA = ctx.enter_context(tc.tile_pool(name="psA", bufs=2, space="PSUM"))
        psO = ctx.enter_context(tc.tile_pool(name="psO", bufs=2, space="PSUM"))
        psT = ctx.enter_context(tc.tile_pool(name="psT", bufs=2, space="PSUM"))
        psP = ctx.enter_context(tc.tile_pool(name="psP", bufs=1, space="PSUM"))
        dram = ctx.enter_context(tc.tile_pool(name="dram", bufs=1, space="DRAM"))

        # ---- constants / resident tiles ----------------------------------
        ident16 = const.tile([128, 128], f16)
        make_identity(nc, ident16)
        ident32 = const.tile([128, 128], f32)
        make_identity(nc, ident32)
        iota_t = const.tile([128, 128], f32)
        nc.gpsimd.iota(iota_t[:], pattern=[[1, 128]], base=0,
                       channel_multiplier=0,
                       allow_small_or_imprecise_dtypes=True)

        dst_res = resident.tile([128, TOTCH], f32)
        nc.sync.dma_start(out=dst_res, in_=t_dst[:, :])
        dinvp_res = resident.tile([128, NB], f32)
        nc.sync.dma_start(out=dinvp_res, in_=t_dinvp[:, :])
        dinvr_res = resident.tile([128, NB], f32)
        nc.sync.dma_start(out=dinvr_res, in_=t_dinvr[:, :])
        wpool_res = resident.tile([128, NB, GPC], f16)
        nc.sync.dma_start(out=wpool_res, in_=t_wpool[:, :, :])
        w1p_res = resident.tile([128, Ll, 2, D_H], f16)
        nc.sync.dma_start(out=w1p_res,
                          in_=t_w1p.rearrange("l k p d -> p l k d"))
        w2p_res = resident.tile([128, Ll, 2, D_H], f16)
        nc.sync.dma_start(out=w2p_res,
                          in_=t_w2p.rearrange("l k p d -> p l k d"))
        linw_res = resident.tile([128, KT, D_H], f16)
        nc.sync.dma_start(out=linw_res, in_=t_linw.rearrange("k p d -> p k d"))
        linb_res = resident.tile([128, D_H], f32)
        nc.gpsimd.dma_start(out=linb_res, in_=bcast_row(t_linb[:, :]))
        lng_res = resident.tile([128, Ll, D_H], f16)
        lnb_res = resident.tile([128, Ll, D_H], f16)
        for l in range(Ll):
            nc.gpsimd.dma_start(out=lng_res[:, l, :], in_=bcast_row(t_lng[l:l + 1, :]))
            nc.gpsimd.dma_start(out=lnb_res[:, l, :], in_=bcast_row(t_lnb[l:l + 1, :]))
        eps_t = const.tile([128, 1], f32)
        nc.vector.memset(eps_t, LN_EPS)

        # DRAM intermediates, round-major: [round, partition, block%4, ...]
        # so per-round transfers are one 2KB/partition contiguous burst.
        ag_in = [dram.tile([NR * 128, 4 * D_H], f16, name=f"ag_in_{l}")
                 for l in range(Ll)]
        h0T_dram = dram.tile([NR * 128, 4 * 2 * 128], f16, name="h0T_dram")
        tables = [[dram.tile([GROUP_ROWS_L[g], D_H], f16, addr_space="Shared",
                             name=f"table_{l}_{g}") for g in range(NGROUP)]
                  for l in range(Ll)]
        GB4 = (GRP_LO[1] - GRP_LO[0]) // 4

        def emit_allgather(l, g):
            if "nocc" in dbg:
                return
            lo_r = GRP_LO[g] // 4
            hi_r = GRP_LO[g + 1] // 4
            nc.gpsimd.collective_compute(
                "AllGather", mybir.AluOpType.bypass,
                ins=[ag_in[l][lo_r * 128:hi_r * 128, :]], outs=[tables[l][g][:]],
                replica_groups=[list(range(NCORES))],
            )

        # ---- input layer: h0 = relu(x @ linw + b), write hs0 = dinv*h0 ---
        for r in range(NR):
            h0T_stg = work.tile([128, 4, 2, 128], f16, tag="h0Tstg")
            hs_stg = work.tile([128, 4, D_H], f16, tag="hsstg")
            for j in range(4):
                b = 4 * r + j
                xt = work.tile([128, KT, 128], f16, tag="xstage")
                nc.sync.dma_start(out=xt, in_=t_xT[b, :, :])
                ps = psO.tile([128, D_H], f32, tag="outp")
                for k in range(KT):
                    nc.tensor.matmul(ps[:], lhsT=xt[:, k, :],
                                     rhs=linw_res[:, k, :],
                                     start=(k == 0), stop=(k == KT - 1))
                # relu(ps + bias): add bias on DVE, relu on ACT
                tmp = work.tile([128, D_H], f32, tag="lntmp")
                nc.vector.tensor_add(out=tmp[:], in0=ps[:], in1=linb_res[:])
                h0 = work.tile([128, D_H], f32, tag="h0f")
                nc.scalar.activation(out=h0[:], in_=tmp[:],
                                     func=mybir.ActivationFunctionType.Relu)
                trp = psT.tile([128, 256], f32, tag="trp")
                nc.tensor.transpose(out=trp[:, 0:128], in_=h0[:, 0:128],
                                    identity=ident32[:])
                nc.tensor.transpose(out=trp[:, 128:256], in_=h0[:, 128:256],
                                    identity=ident32[:])
                nc.vector.tensor_copy(out=h0T_stg[:, j, :, :], in_=trp[:, 0:256])
                nc.scalar.activation(out=hs_stg[:, j, :], in_=h0[:],
                                     func=mybir.ActivationFunctionType.Identity,
                                     scale=dinvr_res[:, b:b + 1])
            nc.sync.dma_start(out=h0T_dram[r * 128:(r + 1) * 128, :], in_=h0T_stg)
            nc.sync.dma_start(out=ag_in[0][r * 128:(r + 1) * 128, :], in_=hs_stg)
            if 4 * r + 3 in AG_BLOCKS:
                emit_allgather(0, AG_BLOCKS[4 * r + 3])

        pool_ps = psP.tile([GPC, D_H], f32)
        qrr = [0]   # global round-robin counter for gather queue striping

        # ---- layers -------------------------------------------------------
        for l in range(Ll):
            table = tables[l]

            for r in range(NR):
                # idx for 4 rounds loaded in one burst (layer-invariant data,
                # ~5KB/partition per super-round)
                if r % 4 == 0:
                    c0s = int(idx_seg_off[r, 0]) * 8
                    r_end = min(r + 4, NR)
                    c1s = (int(idx_seg_off[r_end, 0]) * 8 if r_end < NR
                           else TOTCH * 8)
                    it_sr = ipool.tile([128, SRCOLS_MAX], i16, tag="idxsr")
                    nc.sync.dma_start(out=it_sr[:, :c1s - c0s],
                                      in_=t_idx[:, c0s:c1s])
                # gather stage for this round: per-group v tiles; dma_gather
                # calls interleaved across groups so the 4 SWDGE queues'
                # desc-gen and DMA drain overlap (ring = 1024 descs).
                vt = {}
                segs = {}
                ogs = {}
                for g in range(NGROUP):
                    seg = int(idx_seg_len[r, g])
                    if seg == 0:
                        continue
                    v = vpool.tile([128, CSEG_G[g], D_H], f16, tag=f"v{g}")
                    if "nogather" in dbg:
                        nc.vector.memset(v[:, :seg, :], 0.25)
                    vt[g] = v
                    segs[g] = seg
                    ogs[g] = int(idx_seg_off[r, g]) * 8 - c0s
                if "nogather" not in dbg:
                    CG = 8    # chunks per dma_gather call (1024-desc ring)
                    maxcalls = max((s + CG - 1) // CG for s in segs.values())
                    for ci in range(maxcalls):
                        for g in sorted(segs):
                            s0 = ci * CG
                            if s0 >= segs[g]:
                                continue
                            sub = min(CG, segs[g] - s0)
                            o = ogs[g]
                            nc.gpsimd.dma_gather(
                                vt[g][:, s0:s0 + sub, :],
                                table[g][:, :],
                                it_sr[:, o + s0 * 8:o + (s0 + sub) * 8],
                                num_idxs=sub * 128,
                                num_idxs_reg=sub * 128,
                                elem_size=D_H,
                                queue_num=qrr[0] % NQUEUES,
                            )
                            qrr[0] += 1

                h0T_rt = work.tile([128, 4, 2, 128], f16, tag="h0Ts")
                nc.sync.dma_start(out=h0T_rt, in_=h0T_dram[r * 128:(r + 1) * 128, :])
                hs_rt = work.tile([128, 4, D_H], f16, tag="hsb")
                nc.sync.dma_start(out=hs_rt, in_=ag_in[l][r * 128:(r + 1) * 128, :])
                if l < Ll - 1:
                    hso_stg = work.tile([128, 4, D_H], f16, tag="hsstg")

                for b in range(4 * r, 4 * r + 4):
                    j = b - 4 * r
                    cb = int(C_b[b])
                    # one-hot for the whole block: [128, cb, 128] fp16
                    oh = ohpool.tile([128, CB_MAX, 128], f16, tag="oh")
                    d0 = int(dst_cols_off[b, 0])
                    dst_sl = dst_res[:, d0:d0 + cb]
                    dst_b = bass.AP(tensor=dst_sl.tensor, offset=dst_sl.offset,
                                    ap=[dst_sl.ap[0], dst_sl.ap[1], [0, 128]])
                    io_sl = iota_t[:, :]
                    iota_b = bass.AP(tensor=io_sl.tensor, offset=io_sl.offset,
                                     ap=[io_sl.ap[0], [0, cb], io_sl.ap[1]])
                    if "nooh" in dbg:
                        nc.vector.memset(oh[:, :cb, :], 0.0)
                    else:
                        nc.vector.tensor_tensor(out=oh[:, :cb, :], in0=iota_b,
                                                in1=dst_b, op=mybir.AluOpType.is_equal)

                    # segment-sum into psum; the self-loop term hs (= dinv*h)
                    # is accumulated with one identity matmul at the end.
                    aps = psA.tile([128, D_H], f32, tag="acc")
                    if "noseg" in dbg:
                        nc.vector.memset(aps[:], 0.125)
                    else:
                        mm = 0
                        for g in range(NGROUP):
                            cbg = int(C_bg[b, g])
                            for jj in range(cbg):
                                ohcol = int(dst_cols_off[b, g]) - d0 + jj
                                vslot = int(vslot_off[b, g]) + jj
                                nc.tensor.matmul(
                                    aps[:], lhsT=oh[:, ohcol, :],
                                    rhs=vt[g][:, vslot, :],
                                    start=(mm == 0), stop=False,
                                )
                                mm += 1
                        nc.tensor.matmul(aps[:], lhsT=ident16[:],
                                         rhs=hs_rt[:, j, :],
                                         start=(mm == 0), stop=True)
                    a_sb = work.tile([128, D_H], f32, tag="asb")
                    nc.scalar.activation(out=a_sb[:], in_=aps[:],
                                         func=mybir.ActivationFunctionType.Identity,
                                         scale=dinvp_res[:, b:b + 1])
                    # aT via PE transpose (fp32 psum; fp16 psum reads are slow)
                    trp = psT.tile([128, 256], f32, tag="trp")
                    nc.tensor.transpose(out=trp[:, 0:128], in_=a_sb[:, 0:128],
                                        identity=ident32[:])
                    nc.tensor.transpose(out=trp[:, 128:256], in_=a_sb[:, 128:256],
                                        identity=ident32[:])
                    aT = work.tile([128, 2, 128], f16, tag="aT")
                    nc.vector.tensor_copy(out=aT[:, :, :], in_=trp[:, 0:256])

                    # out = a @ W1' + h0 @ W2''
                    ops = psO.tile([128, D_H], f32, tag="outp")
                    nc.tensor.matmul(ops[:], lhsT=aT[:, 0, :],
                                     rhs=w1p_res[:, l, 0, :], start=True, stop=False)
                    nc.tensor.matmul(ops[:], lhsT=aT[:, 1, :],
                                     rhs=w1p_res[:, l, 1, :], start=False, stop=False)
                    nc.tensor.matmul(ops[:], lhsT=h0T_rt[:, j, 0, :],
                                     rhs=w2p_res[:, l, 0, :], start=False, stop=False)
                    nc.tensor.matmul(ops[:], lhsT=h0T_rt[:, j, 1, :],
                                     rhs=w2p_res[:, l, 1, :], start=False, stop=True)

                    # LayerNorm + gamma/beta + relu
                    stats = work.tile([128, 6], f32, tag="stats")
                    nc.vector.bn_stats(out=stats[:], in_=ops[:])
                    mv = work.tile([128, 2], f32, tag="mv")
                    nc.vector.bn_aggr(out=mv[:], in_=stats[:])
                    rstd = work.tile([128, 1], f32, tag="rstd")
                    nc.scalar.activation(out=rstd[:], in_=mv[:, 1:2],
                                         func=mybir.ActivationFunctionType.Abs_reciprocal_sqrt,
                                         bias=eps_t[:], scale=1.0)
                    nmr = work.tile([128, 1], f32, tag="nmr")
                    nc.vector.tensor_scalar(out=nmr[:], in0=mv[:, 0:1],
                                            scalar1=rstd[:], scalar2=-1.0,
                                            op0=mybir.AluOpType.mult,
                                            op1=mybir.AluOpType.mult)
                    normed = work.tile([128, D_H], f16, tag="normed")
                    nc.scalar.activation(out=normed[:], in_=ops[:],
                                         func=mybir.ActivationFunctionType.Identity,
                                         bias=nmr[:], scale=rstd[:])
                    # gamma * normed + beta, then relu
                    gb = work.tile([128, D_H], f16, tag="gb")
                    nc.vector.scalar_tensor_tensor(
                        out=gb[:], in0=normed[:], scalar=1.0,
                        in1=lng_res[:, l, :],
                        op0=mybir.AluOpType.mult, op1=mybir.AluOpType.mult)
                    hn = work.tile([128, D_H], f16, tag="hn")
                    nc.vector.tensor_tensor(out=hn[:], in0=gb[:],
                                            in1=lnb_res[:, l, :],
                                            op=mybir.AluOpType.add)
                    hr = work.tile([128, D_H], f16, tag="hr")
                    nc.scalar.activation(out=hr[:], in_=hn[:],
                                         func=mybir.ActivationFunctionType.Relu)

                    if l == Ll - 1:
                        if "nopool" not in dbg:
                            nc.tensor.matmul(pool_ps[:], lhsT=wpool_res[:, b, :],
                                             rhs=hr[:], start=(b == 0),
                                             stop=(b == NB - 1),
                                             skip_group_check=True)
                    else:
                        nc.scalar.activation(out=hso_stg[:, j, :], in_=hr[:],
                                             func=mybir.ActivationFunctionType.Identity,
                                             scale=dinvr_res[:, b:b + 1])

                if l < Ll - 1:
                    nc.sync.dma_start(out=ag_in[l + 1][r * 128:(r + 1) * 128, :], in_=hso_stg)
                    if 4 * r + 3 in AG_BLOCKS:
                        emit_allgather(l + 1, AG_BLOCKS[4 * r + 3])

        # ---- head ---------------------------------------------------------
        pooled = work.tile([GPC, D_H], f32, tag="pooled")
        if "nopool" in dbg:
            nc.vector.memset(pooled[:], 0.5)
        else:
            nc.vector.tensor_copy(out=pooled[:], in_=pool_ps[:])

        def head_mm(z, kdim, wtile_list, btile, relu, outdim):
            # z: [GPC, kdim] fp32 sbuf -> out [GPC, outdim] fp32
            trp2 = psT.tile([128, 256], f32, tag="trp")
            zT = work.tile([128, (kdim + 127) // 128, GPC], f32, tag="zT")
            for k in range((kdim + 127) // 128):
                kk = min(128, kdim - k * 128)
                nc.tensor.transpose(out=trp2[:kk, k * 128:k * 128 + GPC],
                                    in_=z[:, k * 128:k * 128 + kk],
                                    identity=ident32[:GPC, :GPC])
                nc.vector.tensor_copy(out=zT[:kk, k, :],
                                      in_=trp2[:kk, k * 128:k * 128 + GPC])
            ps = psO.tile([GPC, max(outdim, 1)], f32, tag="headp", bufs=1)
            nk = (kdim + 127) // 128
            for k in range(nk):
                kk = min(128, kdim - k * 128)
                nc.tensor.matmul(ps[:], lhsT=zT[:kk, k, :], rhs=wtile_list[k][:kk, :],
                                 start=(k == 0), stop=(k == nk - 1))
            zo = work.tile([GPC, outdim], f32, tag="zo")
            nc.vector.tensor_add(out=zo[:], in0=ps[:], in1=btile[:GPC, :])
            if relu:
                zr = work.tile([GPC, outdim], f32, tag="zr")
                nc.scalar.activation(out=zr[:], in_=zo[:],
                                     func=mybir.ActivationFunctionType.Relu)
                return zr
            return zo

        c1w_t = resident.tile([128, 2, D_H], f32)
        nc.sync.dma_start(out=c1w_t, in_=t_c1w.rearrange("k p d -> p k d"))
        c1b_t = resident.tile([128, D_H], f32)
        nc.gpsimd.dma_start(out=c1b_t, in_=bcast_row(t_c1b[:, :]))
        c2w_t = resident.tile([128, 2, D_H // 2], f32)
        nc.sync.dma_start(out=c2w_t, in_=t_c2w.rearrange("k p d -> p k d"))
        c2b_t = resident.tile([128, D_H // 2], f32)
        nc.gpsimd.dma_start(out=c2b_t, in_=bcast_row(t_c2b[:, :]))
        c3w_t = resident.tile([128, 1], f32)
        nc.sync.dma_start(out=c3w_t, in_=t_c3w[:, :])
        c3b_t = resident.tile([128, 1], f32)
        nc.gpsimd.dma_start(out=c3b_t, in_=bcast_row(t_c3b[:, :]))

        z1 = head_mm(pooled, D_H, [c1w_t[:, 0, :], c1w_t[:, 1, :]], c1b_t, True, D_H)
        z2 = head_mm(z1, D_H, [c2w_t[:, 0, :], c2w_t[:, 1, :]], c2b_t, True, D_H // 2)
        z3 = head_mm(z2, D_H // 2, [c3w_t], c3b_t, False, 1)
        nc.sync.dma_start(out=t_y[:, :], in_=z3[:])

    nc.compile()
    return nc


# ---------------------------------------------------------------------------
# entry point
# ---------------------------------------------------------------------------

_CACHE = {}


def kernel(**inputs):
    global LAST_EXEC_NS
    trace = bool(os.environ.get("BASS_TRACE"))
    if trace:
        _maybe_register_ntff_hook()

    in_maps, meta = preprocess(**inputs)
    ckey = ("k", meta["NP"], meta["TOTCH"])
    if ckey not in _CACHE:
        _CACHE[ckey] = build_kernel(meta)
    nc = _CACHE[ckey]

    res = run_bass_kernel_spmd(nc, in_maps, core_ids=list(range(NCORES)),
                               trace=trace)
    LAST_EXEC_NS = res.exec_time_ns
    y = np.concatenate([res.results[c]["y"].reshape(-1) for c in range(NCORES)])
    return y.astype(np.float32)

